# revision 45
# baseline (speedup 1.0000x reference)
"""Trainium2 Bass kernel for nn_AggregateConcatenate.

out[b] = concat([masked {mean,max,min,std} of tanh-MLP_agg(x_b) over the valid
prefix, tanh-MLP_adj(x_b)], axis=1)

Sharding: data-parallel over batch B across 8 NeuronCores (4 bags/core), with
bags clustered by length into slots so all 8 cores run one SPMD program.

Per-core dataflow (matmuls in float32r: full PE rate at N>=256):
  - mm1 feature-major: h^T[h, tok] = W1T.T @ x^T   (x pre-transposed on host)
  - ELU(pre) = max(pre, min(exp(pre) - 1, 0)): exp on ScalarE (bias fused),
    min + max-combine on VectorE; one h-tile per PSUM group with 4 groups of
    lookahead so transient VectorE lag never stalls the PE.
  - mm2 token-major: q[tok, a] = hT.T @ W2T  -> adjacent output rows DMA out
    contiguously; the ragged reductions see tokens on partitions.
  - masked sum / sum-of-squares via PE matmuls with per-(bag, tile) mask
    columns as the stationary operand, accumulated in PSUM across the whole
    kernel; max/min via VectorE accumulators (direct for tiles full for every
    core's bag in the slot, masked candidates for boundary tiles), reduced
    over partitions at the end with PE transposes.
  - agg work is trimmed to 128-token granularity per slot; slots process
    longest-first so the final chunk's adj phase hides the stats/finalize
    drain; std-row sqrt runs on the host so the ACT exp/tanh table set stays
    resident for the whole kernel.
"""

import numpy as np

import concourse.bass as bass
import concourse.tile as tile
from concourse import mybir
from concourse.bass_utils import run_bass_kernel_spmd
from concourse.masks import make_identity
from concourse.tile import ScopedClock

B, T, E, H, A = 32, 2048, 512, 1024, 512
NCORES = 8
BPC = B // NCORES          # bags per core
NCH = T // 512             # 512-token chunks per bag
NTT = 4                    # 128-token tiles per chunk
KE = E // 128              # k-tiles for mm1
KH = H // 128              # k-tiles for mm2
NHT = H // 128             # h-tiles (partition tiles of H)
BIG = 30.0                 # ragged padding offset; |tanh| < 1 << BIG

F32 = mybir.dt.float32
F32R = mybir.dt.float32r
AF = mybir.ActivationFunctionType
OP = mybir.AluOpType


class _SplitDrainTileContext(tile.TileContext):
    """TileContext whose exit drain splits sem waits across sync NOPs."""

    WAIT_LIMIT = 1

    def _drain_and_barrier(self, tick_clock, wait_clock):
        drain_inst = self.nc.sync.drain()
        wait_clock.add_sem_waits(
            drain_inst.ins, ScopedClock({None: tick_clock.global_clock})
        )
        si = drain_inst.ins.sync_info
        if si is not None and len(si.on_wait) > self.WAIT_LIMIT:
            waits = list(si.on_wait)
            drain_inst.ins.sync_info = mybir.SyncInfo(
                on_wait=waits[: self.WAIT_LIMIT], on_update=list(si.on_update)
            )
            for i in range(self.WAIT_LIMIT, len(waits), self.WAIT_LIMIT):
                nop = self.nc.sync.nop()
                nop.ins.sync_info = mybir.SyncInfo(
                    on_wait=waits[i : i + self.WAIT_LIMIT], on_update=[]
                )
        self.nc.all_engine_barrier()
        assert self.sems is not None
        popped = self.nc._tile_sem_poison_stack.pop()
        assert popped is self._sem_poison
        # end-of-kernel teardown: clear the semaphores with the fast
        # RANGE_CLEAR only — the full clear_and_free path also emits a
        # gpsimd dma_reset drain (~3us) that sits on the critical tail;
        # the preceding drain + barrier already retired every DMA
        sems = [
            s.num if hasattr(s, "num") else s
            for s in self.sems.allocated().values()
        ]
        for sem_range in bass.compact_to_ranges(sems):
            self.nc.gpsimd.sem_clear(sem_range)
        self.nc.all_engine_barrier()


def _split_waits(nc, limit: int = 1):
    """Walrus codegen accepts at most one sync wait per TPB instruction.

    Hoist excess waits from any instruction onto injected same-engine NOPs
    placed immediately before it (same-engine program order is preserved, so
    waiting earlier is equivalent).
    """
    uid = [0]
    for f in nc.m.functions:
        for bb in f.blocks:
            new_insts = []
            for ins in bb.instructions:
                si = ins.sync_info
                if si is not None and len(si.on_wait) > limit:
                    waits = list(si.on_wait)
                    keep = waits[:limit]
                    rest = waits[limit:]
                    for j in range(0, len(rest), limit):
                        uid[0] += 1
                        nop = mybir.InstNoOp(
                            name=f"waitnop-{uid[0]}",
                            engine=ins.engine,
                            ins=[],
                            outs=[],
                        )
                        nop.sync_info = mybir.SyncInfo(
                            on_wait=rest[j : j + limit], on_update=[]
                        )
                        new_insts.append(nop)
                    ins.sync_info = mybir.SyncInfo(
                        on_wait=keep, on_update=list(si.on_update)
                    )
                new_insts.append(ins)
            if len(new_insts) != len(bb.instructions):
                bb.instructions = new_insts
    return nc


def _build_program(with_b2: bool, b1_zero: bool, vt: tuple, ft: tuple):
    """vt[b]: valid 128-token tiles for slot b (agg work). ft[b]: tiles that
    are full (all 128 tokens valid) for EVERY core's bag in slot b — those
    accumulate max/min directly without masked candidates. Slots arrive in
    processing order (host sorts longest-first)."""
    nc = bass.Bass()

    # all tensors arrive pre-tiled to the exact SBUF layout so every DMA is
    # a contiguous max-line-size copy
    xt = nc.declare_dram_parameter("xt", [BPC, NCH, 128, KE, 512], F32R, isOutput=False)
    w1t = {}
    w2t = {}
    b1 = {}
    b2 = {}
    for m in ("agg", "adj"):
        # w1 is partition-major with h-tile second so a 256 KB slice (the
        # first h-tile) alone unblocks mm1
        w1t[m] = nc.declare_dram_parameter(
            f"w1t_{m}", [128, NHT, KE, 128], F32R, isOutput=False
        )
        w2t[m] = nc.declare_dram_parameter(
            f"w2t_{m}", [2, 128, KH // 2, A], F32R, isOutput=False
        )
        b1[m] = nc.declare_dram_parameter(f"b1_{m}", [128, NHT], F32, isOutput=False)
        if with_b2:
            b2[m] = nc.declare_dram_parameter(f"b2_{m}", [1, A], F32, isOutput=False)
    maskd = nc.declare_dram_parameter("mask", [128, BPC, 16], F32, isOutput=False)
    negpadd = nc.declare_dram_parameter("negpad", [128, BPC, 16], F32, isOutput=False)
    pospadd = nc.declare_dram_parameter("pospad", [128, BPC, 16], F32, isOutput=False)
    slhsd = nc.declare_dram_parameter("slhs", [128, BPC, 16, BPC], F32R, isOutput=False)
    invnd = nc.declare_dram_parameter("inv_n", [BPC, 1], F32, isOutput=False)
    invnm1d = nc.declare_dram_parameter("inv_nm1", [BPC, 1], F32, isOutput=False)
    novernm1d = nc.declare_dram_parameter("n_over_nm1", [BPC, 1], F32, isOutput=False)
    out = nc.declare_dram_parameter("out", [BPC, 4 + T, A], F32, isOutput=True)

    def rtiles(b, c):
        return max(0, min(NTT, vt[b] - c * NTT))

    with _SplitDrainTileContext(nc) as tc:
        with (
            tc.tile_pool(name="consts", bufs=1) as consts,
            tc.tile_pool(name="accs", bufs=1) as accs,
            tc.tile_pool(name="xin", bufs=2) as xin,
            tc.tile_pool(name="elu", bufs=2) as elu,
            tc.tile_pool(name="ht", bufs=1) as htp,
            tc.tile_pool(name="qp", bufs=1) as qp,
            tc.tile_pool(name="adjo", bufs=3) as adjo,
            tc.tile_pool(name="fin", bufs=1) as fin,
            tc.tile_pool(name="pb1", bufs=4, space="PSUM") as pb1p,
            tc.tile_pool(name="pb2", bufs=2, space="PSUM") as pb2p,
            tc.tile_pool(name="pstat", bufs=1, space="PSUM") as pstat,
        ):
            # ---- DMA preamble, critical path first --------------------------
            # PE needs w1_agg h-tile 0 + the first half of x chunk 0; those two
            # transfers go out first on separate queues with everything else
            # dispatched behind them.
            w1sb = {}
            w2sb = {}
            b1sb = {}
            b2sb = {}
            for m in ("agg", "adj"):
                w1sb[m] = consts.tile([128, NHT, KE, 128], F32R, tag=f"w1_{m}", name=f"w1_{m}")
                w2sb[m] = consts.tile([128, KH, A], F32R, tag=f"w2_{m}", name=f"w2_{m}")
                b1sb[m] = consts.tile([128, NHT], F32, tag=f"b1_{m}", name=f"b1_{m}")

            # the two HWDGE rings (ACT + SP) round-robin at packet granularity,
            # so each ring is kept loaded in deadline order with ~half the
            # early traffic: first MM is gated by w1_agg[ht0] (ACT) + x chunk0
            # (SP); the rest streams in consumption order split across rings.
            nc.scalar.dma_start(out=w1sb["agg"][:, 0], in_=w1t["agg"][:, 0])
            xb00 = xin.tile([128, KE, 512], F32R, tag="xb", name="xb00")
            nc.sync.dma_start(out=xb00, in_=xt[0, 0])

            # dummy fp32 matmuls warm the PE's HAM clock gate (first ~3.4us of
            # PE activity runs at 1.2 GHz otherwise) while the weights stream;
            # warm_sb memset on VectorE right away so the hoisted exp-table
            # load fires during the DMA wait, not before the first real ELU
            dummy = consts.tile([128, 512], F32, tag="dummy", name="dummy")
            nc.vector.memset(dummy, 0.0)
            warm_sb = consts.tile([1, 1], F32, tag="warm", name="warm")
            nc.vector.memset(warm_sb, 0.0)
            nc.scalar.activation(warm_sb, warm_sb, AF.Exp)
            def warm_mm(k):
                for w in range(k):
                    pwarm = pb2p.tile([128, A], F32, tag="pb2", name="pwarm")
                    nc.tensor.matmul(
                        pwarm, lhsT=dummy[:, 0:128], rhs=dummy, start=True, stop=True
                    )

            warm_mm(5)

            # chunk 0 consumes 9.25 MB in its first ~27us — right at the
            # ~358 GB/s HBM limit split evenly between the two HWDGE rings —
            # so both rings are packed in earliest-deadline-first order with
            # fine slices: mm1-agg eats w1_agg at ~290 GB/s from t≈13us,
            # mm1-adj follows, then mm2 needs each w2 all at once.
            nc.scalar.dma_start(out=w1sb["agg"][:, 1:5], in_=w1t["agg"][:, 1:5])
            nc.sync.dma_start(out=w1sb["agg"][:, 5:NHT], in_=w1t["agg"][:, 5:NHT])
            nc.scalar.dma_start(out=w1sb["adj"][:, 0:2], in_=w1t["adj"][:, 0:2])
            nc.sync.dma_start(out=w1sb["adj"][:, 2:4], in_=w1t["adj"][:, 2:4])
            nc.scalar.dma_start(out=w1sb["adj"][:, 4:6], in_=w1t["adj"][:, 4:6])
            nc.sync.dma_start(out=w1sb["adj"][:, 6:8], in_=w1t["adj"][:, 6:8])
            nc.scalar.dma_start(
                out=w2sb["agg"][:, 0 : KH // 2, :], in_=w2t["agg"][0]
            )
            nc.sync.dma_start(
                out=w2sb["agg"][:, KH // 2 : KH, :], in_=w2t["agg"][1]
            )
            nc.scalar.dma_start(
                out=w2sb["adj"][:, KH // 2 : KH, :], in_=w2t["adj"][1]
            )
            nc.sync.dma_start(
                out=w2sb["adj"][:, 0 : KH // 2, :], in_=w2t["adj"][0]
            )

            def gdma(out_ap, in_ap):
                nc.gpsimd.dma_start(out=out_ap, in_=in_ap)

            # identity + accumulator memsets lead the gpsimd queue: they keep
            # its SWDGE DMAs (all needed only ~40us+ in) from stealing HBM
            # bandwidth during the critical chunk-0 window
            ident_f = consts.tile([128, 128], F32, tag="ident_f", name="ident_f")
            make_identity(nc, ident_f)
            acc_max = accs.tile([128, BPC, A], F32, tag="acc_max", name="acc_max")
            nc.gpsimd.memset(acc_max, -1e4)
            acc_min = accs.tile([128, BPC, A], F32, tag="acc_min", name="acc_min")
            nc.gpsimd.memset(acc_min, 1e4)

            for m in ("agg", "adj"):
                gdma(b1sb[m], b1[m][:, :])
                if with_b2:
                    b2sb[m] = consts.tile([1, A], F32, tag=f"b2_{m}", name=f"b2_{m}")
                    gdma(b2sb[m], b2[m][:, :])
            if with_b2:
                ones_col = consts.tile([1, 128], F32, tag="ones", name="ones")
                nc.gpsimd.memset(ones_col, 1.0)

            mask_sb = consts.tile([128, BPC, 16], F32, tag="mask", name="mask")
            gdma(mask_sb, maskd[:, :, :])
            negpad_sb = consts.tile([128, BPC, 16], F32, tag="negpad", name="negpad")
            gdma(negpad_sb, negpadd[:, :, :])
            pospad_sb = consts.tile([128, BPC, 16], F32, tag="pospad", name="pospad")
            gdma(pospad_sb, pospadd[:, :, :])
            slhs_sb = consts.tile([128, BPC, 16, BPC], F32R, tag="slhs", name="slhs")
            gdma(slhs_sb, slhsd[:, :, :, :])
            invn_sb = consts.tile([BPC, 1], F32, tag="invn", name="invn")
            gdma(invn_sb, invnd[:, :])
            invnm1_sb = consts.tile([BPC, 1], F32, tag="invnm1", name="invnm1")
            gdma(invnm1_sb, invnm1d[:, :])
            novernm1_sb = consts.tile([BPC, 1], F32, tag="novernm1", name="novernm1")
            gdma(novernm1_sb, novernm1d[:, :])

            # stats accumulators in PSUM, one matmul accumulation group each
            psum_s = pstat.tile([BPC, A], F32, tag="psum_s", name="psum_s")
            psum_q = pstat.tile([BPC, A], F32, tag="psum_q", name="psum_q")

            # ---- main loops ------------------------------------------------
            pending_stats = []
            n_stat_mm = sum(vt) * 2  # matmuls per stats accumulation group
            stat_i = [0]

            def emit_stats(item):
                pb, pc, pq, pq2 = item
                for tt in range(rtiles(pb, pc)):
                    tg = pc * NTT + tt
                    lhs = slhs_sb[:, pb, tg, :]
                    i = stat_i[0]
                    nc.tensor.matmul(
                        psum_s, lhsT=lhs, rhs=pq[:, tt, :],
                        start=(i == 0), stop=(i == n_stat_mm - 2),
                        skip_group_check=True,
                    )
                    nc.tensor.matmul(
                        psum_q, lhsT=lhs, rhs=pq2[:, tt, :],
                        start=(i == 0), stop=(i == n_stat_mm - 2),
                        skip_group_check=True,
                    )
                    stat_i[0] += 2

            def emit_maxmin_fin(b, defer=None):
                for acc, row, op in ((acc_max, 1, OP.max), (acc_min, 2, OP.min)):
                    pt = pb1p.tile([128, NTT, 128], F32, tag="pb1", name="pt_fin")
                    redt = fin.tile(
                        [128, NTT], F32, tag="redt", name=f"redt_{b}_{row}", bufs=2
                    )
                    for ch in range(NTT):
                        nc.tensor.transpose(
                            pt[:, ch, :], acc[:, b, ch * 128 : (ch + 1) * 128],
                            ident_f,
                        )
                        nc.vector.tensor_reduce(
                            redt[:, ch : ch + 1], pt[:, ch, :],
                            axis=mybir.AxisListType.X, op=op,
                        )
                    prow = pb2p.tile([NTT, 128], F32, tag="pb2", name="prow_fin")
                    nc.tensor.transpose(prow, redt, ident_f)
                    row_sb = fin.tile(
                        [NTT, 128], F32, tag="row", name=f"row_{b}_{row}", bufs=2
                    )
                    nc.scalar.mul(row_sb, prow, 1.0)
                    ap = out[b, row : row + 1, :].rearrange("o (c f) -> (o c) f", c=NTT)
                    if defer is None:
                        nc.gpsimd.dma_start(out=ap, in_=row_sb)
                    else:
                        defer.append((ap, row_sb))

            def emit_meanstd(defer=None):
                # mean / "std" rows (partition = bag); the std row carries the
                # variance — the host takes the sqrt (keeps the sqrt ACT table
                # set off the device entirely)
                mean_sb = fin.tile([BPC, A], F32, tag="mean", name="mean")
                nc.vector.tensor_scalar(mean_sb, psum_s, invn_sb[:, 0:1], None, OP.mult)
                m2_sb = fin.tile([BPC, A], F32, tag="m2", name="m2")
                nc.vector.tensor_tensor(m2_sb, mean_sb, mean_sb, OP.mult)
                s1_sb = fin.tile([BPC, A], F32, tag="s1", name="s1")
                nc.vector.tensor_scalar(s1_sb, psum_q, invnm1_sb[:, 0:1], None, OP.mult)
                s2_sb = fin.tile([BPC, A], F32, tag="s2", name="s2")
                nc.vector.tensor_scalar(s2_sb, m2_sb, novernm1_sb[:, 0:1], None, OP.mult)
                var_sb = fin.tile([BPC, A], F32, tag="var", name="var")
                nc.vector.tensor_tensor(var_sb, s1_sb, s2_sb, OP.subtract)
                pairs = [(out[:, 0, :], mean_sb[0:BPC, :]), (out[:, 3, :], var_sb[0:BPC, :])]
                for ap, sb in pairs:
                    if defer is None:
                        nc.gpsimd.dma_start(out=ap, in_=sb)
                    else:
                        defer.append((ap, sb))

            pending_fin = []
            finalized = [False]
            tail_dmas = []  # final row DMAs dispatched on the ACT HWDGE ring
            for b in range(BPC):
                for c in range(NCH):
                    r = rtiles(b, c)
                    mlps = ("agg", "adj") if r else ("adj",)
                    last_chunk = (b == BPC - 1) and (c == NCH - 1)
                    if b == 0 and c == 0:
                        xb = xb00
                    else:
                        xb = xin.tile([128, KE, 512], F32R, tag="xb", name="xb")
                        nc.sync.dma_start(out=xb, in_=xt[b, c])
                    hts = {}
                    # ---- layer 1 (feature-major) + ELU, both MLPs ----
                    # elu(z) = max(z, min(exp(z) - 1, 0)); exp on ScalarE
                    # (bias fused), min + max-combine on VectorE.  One h-tile
                    # per PSUM group, 4 groups deep, so the PE can run ~3.7us
                    # ahead of the ELU consumers.
                    for m in mlps:
                        n = 512 if m == "adj" else 128 * r
                        htsb = htp.tile(
                            [128, KH, 512], F32R, tag=f"ht_{m}", name=f"ht_{m}"
                        )
                        hts[m] = htsb
                        for ht in range(NHT):
                            if b == 0 and c == 0 and m == "agg" and ht == 1:
                                # chunk 0's w1 bulk is still streaming at the
                                # HBM limit; two fp32 dummies (~2.6us) bridge
                                # the gap so the HAM clock gate stays at 8/8
                                warm_mm(2)
                            pb1 = pb1p.tile([128, 512], F32, tag="pb1", name="pb1")
                            for kt in range(KE):
                                nc.tensor.matmul(
                                    pb1[:, :n],
                                    lhsT=w1sb[m][:, ht, kt, :],
                                    rhs=xb[:, kt, :n],
                                    start=(kt == 0),
                                    stop=(kt == KE - 1),
                                )
                            e_sb = elu.tile([128, 512], F32, tag="e", name="e")
                            if b1_zero:
                                b1col = None
                                nc.scalar.activation(e_sb[:, :n], pb1[:, :n], AF.Exp)
                            else:
                                b1col = b1sb[m][:, ht : ht + 1]
                                nc.scalar.activation(
                                    e_sb[:, :n], pb1[:, :n], AF.Exp, bias=b1col
                                )
                            m_sb = elu.tile([128, 512], F32, tag="m", name="m")
                            nc.vector.tensor_scalar(
                                m_sb[:, :n], e_sb[:, :n],
                                -1.0, 0.0, OP.add, OP.min,
                            )
                            nc.vector.scalar_tensor_tensor(
                                out=htsb[:, ht, :n],
                                in0=pb1[:, :n],
                                scalar=(0.0 if b1_zero else b1col),
                                in1=m_sb[:, :n],
                                op0=OP.add, op1=OP.max,
                            )
                    # ---- layer 2 (token-major) + tanh ----
                    if r:
                        q_sb = qp.tile([128, NTT, A], F32R, tag="q", name="q")
                        q2_sb = qp.tile([128, NTT, A], F32R, tag="q2", name="q2")
                    for mi, m in enumerate(mlps):
                        if m == "adj" and (len(pending_stats) > 1 or r == 0):
                            # drain one deferred stats chunk here: bridges the
                            # adj-ELU latency with PE work that is ready now
                            if pending_stats:
                                emit_stats(pending_stats.pop(0))
                        for tt in range(r if m == "agg" else NTT):
                            pb2 = pb2p.tile([128, A], F32, tag="pb2", name="pb2")
                            for kt in range(KH):
                                nc.tensor.matmul(
                                    pb2,
                                    lhsT=hts[m][:, kt, tt * 128 : (tt + 1) * 128],
                                    rhs=w2sb[m][:, kt, :],
                                    start=(kt == 0),
                                    stop=(kt == KH - 1) and not with_b2,
                                )
                            if with_b2:
                                nc.tensor.matmul(
                                    pb2, lhsT=ones_col, rhs=b2sb[m],
                                    start=False, stop=True,
                                )
                            if m == "adj":
                                adj_sb = adjo.tile([128, A], F32, tag="adj", name="adj")
                                r0 = 4 + c * 512 + tt * 128
                                if last_chunk and tt == NTT - 1:
                                    # final tile: halve the tanh->DMA chain so
                                    # the last flush is 128 KB, not 256 KB
                                    for h0 in (0, 64):
                                        nc.scalar.activation(
                                            adj_sb[h0 : h0 + 64, :],
                                            pb2[h0 : h0 + 64, :], AF.Tanh,
                                        )
                                        nc.sync.dma_start(
                                            out=out[b, r0 + h0 : r0 + h0 + 64, :],
                                            in_=adj_sb[h0 : h0 + 64, :],
                                        )
                                else:
                                    nc.scalar.activation(adj_sb, pb2, AF.Tanh)
                                    nc.sync.dma_start(
                                        out=out[b, r0 : r0 + 128, :], in_=adj_sb
                                    )
                                if last_chunk:
                                    if tt == 1:
                                        while pending_stats:
                                            emit_stats(pending_stats.pop(0))
                                    if tt == 2:
                                        emit_meanstd(tail_dmas)
                                        emit_maxmin_fin(BPC - 1, tail_dmas)
                                        finalized[0] = True
                            else:
                                nc.scalar.activation(q_sb[:, tt, :], pb2, AF.Tanh)
                                tg = c * NTT + tt
                                nc.vector.tensor_tensor(
                                    q2_sb[:, tt, :], q_sb[:, tt, :],
                                    q_sb[:, tt, :], OP.mult,
                                )
                                if tg < ft[b]:
                                    # tile full for every bag in the slot:
                                    # accumulate directly, no masking needed
                                    nc.vector.tensor_tensor(
                                        acc_max[:, b, :], acc_max[:, b, :],
                                        q_sb[:, tt, :], OP.max,
                                    )
                                    nc.vector.tensor_tensor(
                                        acc_min[:, b, :], acc_min[:, b, :],
                                        q_sb[:, tt, :], OP.min,
                                    )
                                else:
                                    cand = elu.tile([128, A], F32, tag="cand", name="cand")
                                    nc.scalar.activation(
                                        cand, q_sb[:, tt, :], AF.Identity,
                                        scale=mask_sb[:, b, tg : tg + 1],
                                        bias=negpad_sb[:, b, tg : tg + 1],
                                    )
                                    nc.vector.tensor_tensor(
                                        acc_max[:, b, :], acc_max[:, b, :], cand,
                                        OP.max,
                                    )
                                    cand2 = elu.tile([128, A], F32, tag="cand2", name="cand2")
                                    nc.scalar.activation(
                                        cand2, q_sb[:, tt, :], AF.Identity,
                                        scale=mask_sb[:, b, tg : tg + 1],
                                        bias=pospad_sb[:, b, tg : tg + 1],
                                    )
                                    nc.vector.tensor_tensor(
                                        acc_min[:, b, :], acc_min[:, b, :], cand2,
                                        OP.min,
                                    )
                        if m == "agg":
                            pending_stats.append((b, c, q_sb, q2_sb))
                    if c == 1 and pending_fin:
                        emit_maxmin_fin(pending_fin.pop(0))
                if b < BPC - 1:
                    pending_fin.append(b)

            while pending_stats:  # only if the in-loop drain never fired
                emit_stats(pending_stats.pop(0))
            if not finalized[0]:
                emit_meanstd(tail_dmas)
                emit_maxmin_fin(BPC - 1, tail_dmas)
            # these land on the scalar queue after the last adj tanh, and on
            # the fast HWDGE ring (the gpsimd SWDGE path adds ~4-10us latency
            # per transfer, which previously gated the final drain)
            for ap, sb in tail_dmas:
                nc.scalar.dma_start(out=ap, in_=sb)
    _split_waits(nc)
    return nc


_PROGRAM_CACHE: dict = {}


def kernel(**inputs) -> np.ndarray:
    x = np.asarray(inputs["x"], np.float32)
    lengths = np.asarray(inputs["padding_lengths"]).astype(np.int64)
    agg_W1 = np.asarray(inputs["agg_W1"], np.float32)
    agg_b1 = np.asarray(inputs["agg_b1"], np.float32)
    agg_W2 = np.asarray(inputs["agg_W2"], np.float32)
    agg_b2 = np.asarray(inputs["agg_b2"], np.float32)
    adj_W1 = np.asarray(inputs["adj_W1"], np.float32)
    adj_b1 = np.asarray(inputs["adj_b1"], np.float32)
    adj_W2 = np.asarray(inputs["adj_W2"], np.float32)
    adj_b2 = np.asarray(inputs["adj_b2"], np.float32)

    with_b2 = bool(np.any(agg_b2) or np.any(adj_b2))
    b1_zero = not (np.any(agg_b1) or np.any(adj_b1))

    # cluster bags by length so short bags share a slot across cores and the
    # agg path can skip invalid 128-token tiles per slot; process slots
    # longest-first so the tail of the program is mostly adj-only work
    perm = np.argsort(lengths, kind="stable")  # slot s holds ranks [8s, 8s+8)
    slot_vt = [
        int(np.ceil(lengths[perm[s * NCORES : (s + 1) * NCORES]].max() / 128))
        for s in range(BPC)
    ]
    slot_ft = [
        int(lengths[perm[s * NCORES : (s + 1) * NCORES]].min() // 128)
        for s in range(BPC)
    ]
    order = sorted(range(BPC), key=lambda s: -slot_vt[s])  # processing order
    vt = tuple(slot_vt[s] for s in order)
    ft = tuple(slot_ft[s] for s in order)
    key = (with_b2, b1_zero, vt, ft)
    if key not in _PROGRAM_CACHE:
        _PROGRAM_CACHE[key] = _build_program(with_b2, b1_zero, vt, ft)
    nc = _PROGRAM_CACHE[key]

    # ---- host-side input prep (pre-tiled to SBUF layouts) -----------------
    # xt[b, c, p, kt, t] = x[b, c*512 + t, kt*128 + p]
    xt = np.ascontiguousarray(
        x.reshape(B, NCH, 512, KE, 128).transpose(0, 1, 4, 3, 2)
    )

    def tile_w1(w):  # [H, E] -> [128, NHT, KE, 128]; [p, ht, kt, h_lo]
        wt = w.T.reshape(KE, 128, NHT, 128).transpose(1, 2, 0, 3)
        return np.ascontiguousarray(wt)

    def tile_w2(w):  # [A, H] -> [2, 128, KH//2, A]; [half, p, kt, a]
        wt = w.T.reshape(2, KH // 2, 128, A).transpose(0, 2, 1, 3)
        return np.ascontiguousarray(wt)

    w1t = {"agg": tile_w1(agg_W1), "adj": tile_w1(adj_W1)}
    w2t = {"agg": tile_w2(agg_W2), "adj": tile_w2(adj_W2)}
    b1 = {
        "agg": np.ascontiguousarray(agg_b1.reshape(NHT, 128).T),
        "adj": np.ascontiguousarray(adj_b1.reshape(NHT, 128).T),
    }
    b2 = {"agg": agg_b2.reshape(1, A), "adj": adj_b2.reshape(1, A)}

    mask = (np.arange(T)[None, :] < lengths[:, None]).astype(np.float32)  # [B, T]
    negpad = (mask - 1.0) * BIG
    pospad = (1.0 - mask) * BIG
    # stationary mask columns for the stats matmuls: [B, 16 tok-tiles, 128, BPC]
    # the output partition is the bag's PROCESSING slot index
    rank_of = np.empty(B, np.int64)
    rank_of[perm] = np.arange(B)
    slot_of = rank_of // NCORES  # sorted-slot index per bag
    inv_order = np.empty(BPC, np.int64)
    inv_order[np.asarray(order)] = np.arange(BPC)
    proc_of = inv_order[slot_of]
    slhs_local = np.zeros((B, 16, 128, BPC), np.float32)
    mask_t = mask.reshape(B, 16, 128)
    for bb in range(B):
        slhs_local[bb, :, :, int(proc_of[bb])] = mask_t[bb]
    n = lengths.astype(np.float64)
    inv_n = (1.0 / n).astype(np.float32).reshape(B, 1)
    inv_nm1 = (1.0 / (n - 1.0)).astype(np.float32).reshape(B, 1)
    n_over_nm1 = (n / (n - 1.0)).astype(np.float32).reshape(B, 1)

    in_maps = []
    core_bags = []  # bag ids per core, in processing order
    for c in range(NCORES):
        sl = perm[c::NCORES][np.asarray(order)]  # processing order for core c
        core_bags.append(sl)
        im = {
            "xt": xt[sl],
            "w1t_agg": w1t["agg"], "w2t_agg": w2t["agg"], "b1_agg": b1["agg"],
            "w1t_adj": w1t["adj"], "w2t_adj": w2t["adj"], "b1_adj": b1["adj"],
            "mask": np.ascontiguousarray(
                mask[sl].reshape(BPC, 16, 128).transpose(2, 0, 1)
            ),
            "negpad": np.ascontiguousarray(
                negpad[sl].reshape(BPC, 16, 128).transpose(2, 0, 1)
            ),
            "pospad": np.ascontiguousarray(
                pospad[sl].reshape(BPC, 16, 128).transpose(2, 0, 1)
            ),
            "slhs": np.ascontiguousarray(slhs_local[sl].transpose(2, 0, 1, 3)),
            "inv_n": inv_n[sl], "inv_nm1": inv_nm1[sl],
            "n_over_nm1": n_over_nm1[sl],
        }
        if with_b2:
            im["b2_agg"] = b2["agg"]
            im["b2_adj"] = b2["adj"]
        in_maps.append(im)

    res = run_bass_kernel_spmd(nc, in_maps, core_ids=list(range(NCORES)))
    out = np.empty((B, 4 + T, A), np.float32)
    for c in range(NCORES):
        out[core_bags[c]] = res.results[c]["out"]
    # the device writes variance into the std row; finish it here
    out[:, 3, :] = np.sqrt(np.maximum(out[:, 3, :], 0.0))
    return out
